# revision 9
# baseline (speedup 1.0000x reference)
"""Trainium2 Bass kernel for BlockUncertaintyTracker (segment_reduce).

Computes, per 4x4 block of a [16,1,2048,2048] image batch:
  - mean over the 16 block elements, averaged over batch
  - 0.9-quantile (= 0.5*(2nd largest + 3rd largest)), averaged over batch
  - EMA update of both stats, then broadcasts the ratio back to full shape.

Sharding: spatial over H across 8 cores (256 image rows / 64 block rows per
core). Every core sees all 16 batch elements for its rows, so no collectives
are needed; EMA buffer slices are contiguous per core.

Pipeline: 4 column chunks (512 cols each) x 2 supergroups (8 batches each).
Within a (chunk, supergroup) unit the 4 groups' data is CONCATENATED along
the free dim so every DVE merge op runs at the full 2048/1024/512 widths
(amortizing fixed per-op cost) while chunking lets each chunk's 8 MiB output
write overlap the next chunk's compute.

Engines: Act casts f32->f16 and does most even/odd deinterleaves; Pool
(gpsimd) takes a share of deints; DVE runs the 26-op sorted-3 merge network
in f16 2x mode; PE accumulates block sums AND the quantile stat with f16
matmuls (ones lhsT folds batch pairs + duplicates rows for the output
layout); per-chunk tail computes the EMA ratio and one broadcast-source DMA
replicates it to all batches.
"""

import os

import numpy as np

# ---- problem constants (hardcoded; kernel.py must be self-contained) ----
B = 16          # batch
H = 2048
W = 2048
BS = 4          # block size
NCORES = 8
HS = H // NCORES            # 256 rows per core
NBH = HS // BS              # 64 block rows per core
NBW = W // BS               # 512 block cols
ROWS = B * HS               # 4096 rows in a per-core slab
NCH = 4                     # column chunks
CW = W // NCH               # 512 image cols per chunk
CB = CW // BS               # 128 block cols per chunk
NSG = 2                     # supergroups (8 batches each)
GPS = 4                     # groups per supergroup (2 batches per group)
DECAY = 0.99
ALPHA = 0.1
EPS = 1e-5
C_MEAN = (1.0 - DECAY) / (BS * BS * B)    # fold mean-over-16-elems and batch
C_QUANT = (1.0 - DECAY) * 0.5 / B         # fold 0.5*(m2+m3) and batch mean

_CACHE = {}


def _split_multi_waits(nc):
    """This walrus build encodes at most ONE sync wait per instruction.
    Tile attaches several. Hoist excess waits onto same-engine NOPs placed
    immediately before the owning instruction (same engine stream => same
    semantics)."""
    import concourse.mybir as mybir

    plans = []  # (inst_name, extra_waits)
    for f in nc.m.functions:
        for bb in f.blocks:
            for inst in bb.instructions:
                si = getattr(inst, "sync_info", None)
                waits = list(si.on_wait) if (si and si.on_wait) else []
                if len(waits) > 1:
                    si.on_wait = [waits[-1]]
                    plans.append((inst.name, waits[:-1]))

    if not plans:
        return

    nop_for = {}
    stray = set()
    for iname, extra in plans:
        nops = []
        for w in extra:
            nop = nc.engines[nc.inst_map[iname].engine].nop(nofuse=True).ins
            nop.sync_info = mybir.SyncInfo(on_wait=[w], on_update=[])
            nops.append(nop)
            stray.add(nop.name)
        nop_for[iname] = nops

    for f in nc.m.functions:
        for bb in f.blocks:
            out = []
            changed = False
            for inst in bb.instructions:
                if inst.name in stray:
                    changed = True
                    continue
                if inst.name in nop_for:
                    out.extend(nop_for[inst.name])
                    changed = True
                out.append(inst)
            if changed:
                bb.instructions = out


def _build():
    """Builds the single-core Bass program (SPMD across 8 cores)."""
    from contextlib import ExitStack

    import concourse.bass as bass
    import concourse.mybir as mybir
    import concourse.tile as tile

    f32 = mybir.dt.float32
    f16 = mybir.dt.float16
    MAX = mybir.AluOpType.max
    MIN = mybir.AluOpType.min
    MULT = mybir.AluOpType.mult
    ADD = mybir.AluOpType.add

    nc = bass.Bass("TRN2", target_bir_lowering=False, debug=False)

    x = nc.dram_tensor("x", [ROWS, W], f32, kind="ExternalInput").ap()
    ee = nc.dram_tensor("ee", [NBH, NBW], f32, kind="ExternalInput").ap()
    eq = nc.dram_tensor("eq", [NBH, NBW], f32, kind="ExternalInput").ap()
    # ones16[p, m] = (p % 64 == m // 2): batch-pair fold + row duplication
    ones16 = nc.dram_tensor("ones16", [128, 128], f16, kind="ExternalInput").ap()
    y = nc.dram_tensor("y", [ROWS, W], f32, kind="ExternalOutput").ap()

    # input row = b*256 + i*4 + r with b = sg*8 + gg*2 + b2.
    # per (ch, sg, r): [128=(b2 i), (gg w)=2048] -- 4 groups concatenated.
    xr = x.rearrange(
        "(sg gg b2 i r) (ch w) -> ch sg r (b2 i) gg w",
        sg=NSG, gg=GPS, b2=2, i=NBH, r=BS, ch=NCH, w=CW,
    )
    # output row = b*256 + i*4 + h*2 + r2; per ch: [128=(i r2), b, h, w]
    yr = y.rearrange(
        "(b i h r2) (ch w) -> ch h r2 i b w",
        b=B, i=NBH, h=2, r2=2, ch=NCH, w=CW,
    )
    # EMA buffers per chunk: [64, CB] -> broadcast row pairs -> [128, CB]
    eer = ee.rearrange("i (ch j) -> ch i j", ch=NCH)
    eqr = eq.rearrange("i (ch j) -> ch i j", ch=NCH)

    with tile.TileContext(nc) as tc, ExitStack() as ctx:
        pool = ctx.enter_context(tc.tile_pool(name="work", bufs=1))
        ppool = ctx.enter_context(tc.tile_pool(name="acc", bufs=1, space="PSUM"))

        ones_sb = pool.tile([128, 128], f16, tag="ones")
        nc.sync.dma_start(ones_sb[:, :], ones16)

        def tt(dst, a, bb, op):
            nc.vector.tensor_tensor(dst, a, bb, op)

        NMM = 16  # sum matmuls per (ch, sg) unit

        for ch in range(NCH):
            psum_s = ppool.tile([128, CW], f32, tag="ps", bufs=2, name=f"ps{ch}")
            psum_q = ppool.tile([128, CW], f32, tag="pq", bufs=2, name=f"pq{ch}")

            ee_sb = pool.tile([128, CB], f32, tag="eesb", bufs=2, name=f"ee{ch}")
            nc.gpsimd.dma_start(
                ee_sb[:, :], eer[ch].unsqueeze(1).broadcast_to((NBH, 2, CB))
            )
            eq_sb = pool.tile([128, CB], f32, tag="eqsb", bufs=2, name=f"eq{ch}")
            nc.gpsimd.dma_start(
                eq_sb[:, :], eqr[ch].unsqueeze(1).broadcast_to((NBH, 2, CB))
            )

            for sg in range(NSG):
                u = f"{ch}_{sg}"
                # ---- load: 4 row-phase tiles, 4 groups concatenated ----
                rts = []
                for r in range(BS):
                    rt = pool.tile(
                        [128, 4 * CW], f32, tag=f"in{r}", bufs=2, name=f"rt{r}_{u}"
                    )
                    nc.sync.dma_start(
                        rt.rearrange("p (g w) -> p g w", g=GPS), xr[ch, sg, r]
                    )
                    rts.append(rt)

                # ---- cast to f16 on Act ----
                bts = []
                for r in range(BS):
                    bt = pool.tile(
                        [128, 4 * CW], f16, tag=f"bc{r}", bufs=2, name=f"bt{r}_{u}"
                    )
                    nc.scalar.copy(bt[:, :], rts[r][:, :])
                    bts.append(bt)
                b0, b1, b2_, b3 = bts

                # ---- vertical sorted-3 across the 4 block rows (DVE) ----
                def vt(name, tag="vq", bufs=1):
                    return pool.tile([128, 4 * CW], f16, tag=tag, bufs=bufs,
                                     name=f"{name}_{u}")

                v1 = vt("v1", "v1")
                tt(v1[:, :], b0[:, :], b1[:, :], MAX)
                w1v = vt("w1v", "w1v")
                tt(w1v[:, :], b0[:, :], b1[:, :], MIN)
                v2 = vt("v2", "v2")
                tt(v2[:, :], b2_[:, :], b3[:, :], MAX)
                w2v = vt("w2v", "w2v")
                tt(w2v[:, :], b2_[:, :], b3[:, :], MIN)
                m = vt("m", "m", bufs=2)
                tt(m[:, :], v1[:, :], v2[:, :], MAX)
                t1 = vt("t1", "t1")
                tt(t1[:, :], v1[:, :], v2[:, :], MIN)
                t2 = vt("t2", "t2")
                tt(t2[:, :], w1v[:, :], w2v[:, :], MAX)
                s2 = vt("s2", "s2", bufs=2)
                tt(s2[:, :], t1[:, :], t2[:, :], MAX)
                t3 = vt("t3", "t3", bufs=2)
                tt(t3[:, :], t1[:, :], t2[:, :], MIN)

                # ---- sum path: 16 f16 matmuls accumulate column sums ----
                # v1+w1v+v2+w2v == b0+b1+b2+b3 elementwise (pair min/max
                # preserves sums), so the PE accumulates exact f16 block
                # column-sums with batch-pair fold + row duplication.
                k0 = sg * NMM
                for ti, vtile in enumerate((v1, w1v, v2, w2v)):
                    for g in range(GPS):
                        k = k0 + ti * GPS + g
                        nc.tensor.matmul(
                            psum_s[:, :], lhsT=ones_sb[:, :],
                            rhs=vtile[:, g * CW : (g + 1) * CW],
                            start=(k == 0), stop=(k == NSG * NMM - 1),
                        )

                # ---- A level: merge column pairs (4 -> 2 per block) ----
                # deint planes m, s2, t3 into even/odd (Act + Pool)
                def deint(src, w_out, par, tag, name, eng):
                    v = src.rearrange("p (g j two) -> p g j two", g=GPS, two=2)
                    t = pool.tile([128, w_out], f16, tag=tag, bufs=1,
                                  name=name + ("e" if par == 0 else "o"))
                    cp = getattr(eng, "copy", None) or eng.tensor_copy
                    cp(t.rearrange("p (g j) -> p g j", g=GPS), v[:, :, :, par])
                    return t

                HW2 = 2 * CW
                me = deint(m, HW2, 0, "me", f"me_{u}", nc.scalar)
                mo = deint(m, HW2, 1, "mo", f"mo_{u}", nc.scalar)
                s2e = deint(s2, HW2, 0, "s2e", f"s2e_{u}", nc.scalar)
                s2o = deint(s2, HW2, 1, "s2o", f"s2o_{u}", nc.scalar)
                t3e = deint(t3, HW2, 0, "t3e", f"t3e_{u}", nc.gpsimd)
                t3o = deint(t3, HW2, 1, "t3o", f"t3o_{u}", nc.gpsimd)

                def mid(name):
                    return pool.tile([128, HW2], f16, tag=name, bufs=1,
                                     name=f"{name}_{u}")

                p1 = mid("p1")
                tt(p1[:, :], me[:, :], mo[:, :], MAX)
                u1 = mid("u1")
                tt(u1[:, :], me[:, :], mo[:, :], MIN)
                u2 = mid("u2")
                tt(u2[:, :], s2e[:, :], s2o[:, :], MAX)
                p2 = mid("p2")
                tt(p2[:, :], u1[:, :], u2[:, :], MAX)
                w2 = mid("w2")
                tt(w2[:, :], me[:, :], s2o[:, :], MIN)
                w3 = mid("w3")
                tt(w3[:, :], s2e[:, :], mo[:, :], MIN)
                w4 = mid("w4")
                tt(w4[:, :], w2[:, :], w3[:, :], MAX)
                w1 = mid("w1")
                tt(w1[:, :], t3e[:, :], t3o[:, :], MAX)
                p3 = mid("p3")
                tt(p3[:, :], w1[:, :], w4[:, :], MAX)

                # ---- B level: final merge (2 -> 1 per block) ----
                def deintB(src, par, tag, name, eng):
                    v = src.rearrange("p (g j two) -> p g j two", g=GPS, two=2)
                    t = pool.tile([128, CW], f16, tag=tag, bufs=1,
                                  name=name + ("e" if par == 0 else "o"))
                    cp = getattr(eng, "copy", None) or eng.tensor_copy
                    cp(t.rearrange("p (g j) -> p g j", g=GPS), v[:, :, :, par])
                    return t

                p1e = deintB(p1, 0, "p1e", f"p1e_{u}", nc.scalar)
                p1o = deintB(p1, 1, "p1o", f"p1o_{u}", nc.scalar)
                p2e = deintB(p2, 0, "p2e", f"p2e_{u}", nc.scalar)
                p2o = deintB(p2, 1, "p2o", f"p2o_{u}", nc.scalar)
                p3e = deintB(p3, 0, "p3e", f"p3e_{u}", nc.scalar)
                p3o = deintB(p3, 1, "p3o", f"p3o_{u}", nc.scalar)

                def small(name):
                    return pool.tile([128, CW], f16, tag=name, bufs=1,
                                     name=f"{name}_{u}")

                z1 = small("z1")
                tt(z1[:, :], p1e[:, :], p1o[:, :], MIN)
                z2 = small("z2")
                tt(z2[:, :], p2e[:, :], p2o[:, :], MAX)
                c2 = small("c2")
                tt(c2[:, :], z1[:, :], z2[:, :], MAX)
                z4 = small("z4")
                tt(z4[:, :], p1e[:, :], p2o[:, :], MIN)
                z5 = small("z5")
                tt(z5[:, :], p2e[:, :], p1o[:, :], MIN)
                z6 = small("z6")
                tt(z6[:, :], z4[:, :], z5[:, :], MAX)
                z3 = small("z3")
                tt(z3[:, :], p3e[:, :], p3o[:, :], MAX)
                c3 = small("c3")
                tt(c3[:, :], z3[:, :], z6[:, :], MAX)

                # quantile stat: psum_q += fold(c2) + fold(c3) (the c2+c3
                # add is folded into two PE accumulation matmuls)
                nc.tensor.matmul(
                    psum_q[:, :], lhsT=ones_sb[:, :], rhs=c2[:, :],
                    start=(sg == 0), stop=False,
                )
                nc.tensor.matmul(
                    psum_q[:, :], lhsT=ones_sb[:, :], rhs=c3[:, :],
                    start=False, stop=(sg == NSG - 1),
                )

            # ---- per-chunk tail: fold, EMA update, ratio, broadcast ----
            def tail_tile(name, wdt=CB, dt=f32):
                return pool.tile([128, wdt], dt, tag=name, bufs=2,
                                 name=f"{name}_{ch}")

            # S[p, j] = sum over the 4 columns of each block
            S = tail_tile("S")
            nc.vector.tensor_reduce(
                S[:, :], psum_s.rearrange("p (j c) -> p j c", c=BS),
                mybir.AxisListType.X, ADD,
            )
            # qf[p, j] = sum over the 4 concatenated groups
            qf = tail_tile("qf")
            nc.vector.tensor_reduce(
                qf[:, :], psum_q.rearrange("p (g j) -> p j g", g=GPS),
                mybir.AxisListType.X, ADD,
            )

            ee2 = tail_tile("ee2")
            nc.scalar.activation(
                ee2[:, :], ee_sb[:, :],
                mybir.ActivationFunctionType.Copy, bias=EPS, scale=DECAY,
            )
            eq2 = tail_tile("eq2")
            nc.scalar.activation(
                eq2[:, :], eq_sb[:, :],
                mybir.ActivationFunctionType.Copy, bias=0.0, scale=DECAY,
            )

            den = tail_tile("den")
            nc.vector.scalar_tensor_tensor(
                den[:, :], S[:, :], C_MEAN, ee2[:, :], op0=MULT, op1=ADD
            )
            num = tail_tile("num")
            nc.vector.scalar_tensor_tensor(
                num[:, :], qf[:, :], C_QUANT, eq2[:, :], op0=MULT, op1=ADD
            )
            rec = tail_tile("rec")
            nc.vector.reciprocal(rec[:, :], den[:, :])
            uu = tail_tile("uu")
            nc.vector.tensor_tensor(uu[:, :], num[:, :], rec[:, :], MULT)

            # expand x4 along columns on Act: u4[p, j*4 + c] = uu[p, j]
            u4 = tail_tile("u4", CW)
            u4v = u4.rearrange("p (j c) -> p j c", c=BS)
            for c in range(BS):
                nc.scalar.copy(u4v[:, :, c], uu[:, :])

            # broadcast-source DMAs per (h, r2): dst/src both [64, B, CW]
            u4r = u4.rearrange("(i r2) w -> r2 i w", r2=2)
            for r2 in range(2):
                u4b = u4r[r2].unsqueeze(1).broadcast_to((NBH, B, CW))
                for h in range(2):
                    nc.gpsimd.dma_start(yr[ch, h, r2], u4b)

    _split_multi_waits(nc)
    return nc


def _get_nc():
    if "nc" not in _CACHE:
        _CACHE["nc"] = _build()
    return _CACHE["nc"]


def kernel(current_errors, ema_errors, ema_quantile):
    from concourse.bass_utils import run_bass_kernel_spmd

    x = np.asarray(current_errors, dtype=np.float32).reshape(B, H, W)
    ee = np.asarray(ema_errors, dtype=np.float32).reshape(H // BS, W // BS)
    eq = np.asarray(ema_quantile, dtype=np.float32).reshape(H // BS, W // BS)

    # ones16[p, m] == 1 iff p % 64 == m // 2 (batch-pair fold + row dup)
    ones16 = np.zeros((128, 128), dtype=np.float16)
    p = np.arange(128)
    ones16[p, (p % NBH) * 2] = 1.0
    ones16[p, (p % NBH) * 2 + 1] = 1.0

    in_maps = []
    for k in range(NCORES):
        xs = np.ascontiguousarray(x[:, k * HS : (k + 1) * HS, :]).reshape(ROWS, W)
        ees = np.ascontiguousarray(ee[k * NBH : (k + 1) * NBH, :])
        eqs = np.ascontiguousarray(eq[k * NBH : (k + 1) * NBH, :])
        in_maps.append({"x": xs, "ee": ees, "eq": eqs, "ones16": ones16})

    nc = _get_nc()
    trace = bool(int(os.environ.get("KERNEL_TRACE", "0")))
    try:
        res = run_bass_kernel_spmd(
            nc, in_maps, core_ids=list(range(NCORES)), trace=trace
        )
    except Exception:
        # transient device state (e.g. NRT_EXEC_UNIT_UNRECOVERABLE) — retry once
        res = run_bass_kernel_spmd(
            nc, in_maps, core_ids=list(range(NCORES)), trace=trace
        )
    _CACHE["last_results"] = res

    out = np.empty((B, 1, H, W), dtype=np.float32)
    for k in range(NCORES):
        out[:, 0, k * HS : (k + 1) * HS, :] = res.results[k]["y"].reshape(B, HS, W)
    return out


# revision 10
# speedup vs baseline: 1.4712x; 1.4712x over previous
"""Trainium2 Bass kernel for BlockUncertaintyTracker (segment_reduce).

Computes, per 4x4 block of a [16,1,2048,2048] image batch:
  - mean over the 16 block elements, averaged over batch
  - 0.9-quantile (= 0.5*(2nd largest + 3rd largest)), averaged over batch
  - EMA update of both stats, then broadcasts the ratio back to full shape.

Sharding: spatial over H across 8 cores (256 image rows / 64 block rows per
core). Every core sees all 16 batch elements for its rows, so no collectives
are needed; EMA buffer slices are contiguous per core.

Pipeline: 4 column chunks (512 cols each) x 2 supergroups (8 batches each).
Within a (chunk, supergroup) unit the 4 groups' data is CONCATENATED along
the free dim so every DVE merge op runs at the full 2048/1024/512 widths
(amortizing fixed per-op cost) while chunking lets each chunk's 8 MiB output
write overlap the next chunk's compute.

Engines: Act casts f32->f16 and does most even/odd deinterleaves; Pool
(gpsimd) takes a share of deints; DVE runs the 26-op sorted-3 merge network
in f16 2x mode; PE accumulates block sums AND the quantile stat with f16
matmuls (ones lhsT folds batch pairs + duplicates rows for the output
layout); per-chunk tail computes the EMA ratio and one broadcast-source DMA
replicates it to all batches.
"""

import os

import numpy as np

# ---- problem constants (hardcoded; kernel.py must be self-contained) ----
B = 16          # batch
H = 2048
W = 2048
BS = 4          # block size
NCORES = 8
HS = H // NCORES            # 256 rows per core
NBH = HS // BS              # 64 block rows per core
NBW = W // BS               # 512 block cols
ROWS = B * HS               # 4096 rows in a per-core slab
NCH = 4                     # column chunks
CW = W // NCH               # 512 image cols per chunk
CB = CW // BS               # 128 block cols per chunk
NSG = 2                     # supergroups (8 batches each)
GPS = 4                     # groups per supergroup (2 batches per group)
DECAY = 0.99
ALPHA = 0.1
EPS = 1e-5
C_MEAN = (1.0 - DECAY) / (BS * BS * B)    # fold mean-over-16-elems and batch
C_QUANT = (1.0 - DECAY) * 0.5 / B         # fold 0.5*(m2+m3) and batch mean

_CACHE = {}


def _split_multi_waits(nc):
    """This walrus build encodes at most ONE sync wait per instruction.
    Tile attaches several. Hoist excess waits onto same-engine NOPs placed
    immediately before the owning instruction (same engine stream => same
    semantics)."""
    import concourse.mybir as mybir

    plans = []  # (inst_name, extra_waits)
    for f in nc.m.functions:
        for bb in f.blocks:
            for inst in bb.instructions:
                si = getattr(inst, "sync_info", None)
                waits = list(si.on_wait) if (si and si.on_wait) else []
                if len(waits) > 1:
                    si.on_wait = [waits[-1]]
                    plans.append((inst.name, waits[:-1]))

    if not plans:
        return

    nop_for = {}
    stray = set()
    for iname, extra in plans:
        nops = []
        for w in extra:
            nop = nc.engines[nc.inst_map[iname].engine].nop(nofuse=True).ins
            nop.sync_info = mybir.SyncInfo(on_wait=[w], on_update=[])
            nops.append(nop)
            stray.add(nop.name)
        nop_for[iname] = nops

    for f in nc.m.functions:
        for bb in f.blocks:
            out = []
            changed = False
            for inst in bb.instructions:
                if inst.name in stray:
                    changed = True
                    continue
                if inst.name in nop_for:
                    out.extend(nop_for[inst.name])
                    changed = True
                out.append(inst)
            if changed:
                bb.instructions = out


def _build():
    """Builds the single-core Bass program (SPMD across 8 cores)."""
    from contextlib import ExitStack

    import concourse.bass as bass
    import concourse.mybir as mybir
    import concourse.tile as tile

    f32 = mybir.dt.float32
    f16 = mybir.dt.float16
    MAX = mybir.AluOpType.max
    MIN = mybir.AluOpType.min
    MULT = mybir.AluOpType.mult
    ADD = mybir.AluOpType.add

    nc = bass.Bass("TRN2", target_bir_lowering=False, debug=False)

    x = nc.dram_tensor("x", [ROWS, W], f32, kind="ExternalInput").ap()
    ee = nc.dram_tensor("ee", [NBH, NBW], f32, kind="ExternalInput").ap()
    eq = nc.dram_tensor("eq", [NBH, NBW], f32, kind="ExternalInput").ap()
    # ones16[p, m] = (p % 64 == m // 2): batch-pair fold + row duplication
    ones16 = nc.dram_tensor("ones16", [128, 128], f16, kind="ExternalInput").ap()
    y = nc.dram_tensor("y", [ROWS, W], f32, kind="ExternalOutput").ap()

    # input row = b*256 + i*4 + r with b = sg*8 + gg*2 + b2.
    # per (ch, sg, r): [128=(b2 i), (gg w)=2048] -- 4 groups concatenated.
    xr = x.rearrange(
        "(sg gg b2 i r) (ch w) -> ch sg r (b2 i) gg w",
        sg=NSG, gg=GPS, b2=2, i=NBH, r=BS, ch=NCH, w=CW,
    )
    # output row = b*256 + i*4 + h*2 + r2; per ch: [128=(i r2), b, h, w]
    yr = y.rearrange(
        "(b i h r2) (ch w) -> ch h r2 i b w",
        b=B, i=NBH, h=2, r2=2, ch=NCH, w=CW,
    )
    # EMA buffers per chunk: [64, CB] -> broadcast row pairs -> [128, CB]
    eer = ee.rearrange("i (ch j) -> ch i j", ch=NCH)
    eqr = eq.rearrange("i (ch j) -> ch i j", ch=NCH)

    with tile.TileContext(nc) as tc, ExitStack() as ctx:
        pool = ctx.enter_context(tc.tile_pool(name="work", bufs=1))
        ppool = ctx.enter_context(tc.tile_pool(name="acc", bufs=1, space="PSUM"))

        ones_sb = pool.tile([128, 128], f16, tag="ones")
        nc.sync.dma_start(ones_sb[:, :], ones16)

        def tt(dst, a, bb, op):
            nc.vector.tensor_tensor(dst, a, bb, op)

        NMM = 16  # sum matmuls per (ch, sg) unit

        for ch in range(NCH):
            psum_s = ppool.tile([128, CW], f32, tag="ps", bufs=2, name=f"ps{ch}")
            psum_q = ppool.tile([128, CW], f32, tag="pq", bufs=2, name=f"pq{ch}")

            ee_sb = pool.tile([128, CB], f32, tag="eesb", bufs=2, name=f"ee{ch}")
            nc.gpsimd.dma_start(
                ee_sb[:, :], eer[ch].unsqueeze(1).broadcast_to((NBH, 2, CB))
            )
            eq_sb = pool.tile([128, CB], f32, tag="eqsb", bufs=2, name=f"eq{ch}")
            nc.gpsimd.dma_start(
                eq_sb[:, :], eqr[ch].unsqueeze(1).broadcast_to((NBH, 2, CB))
            )

            for sg in range(NSG):
                u = f"{ch}_{sg}"
                # ---- load: 4 row-phase tiles, 4 groups concatenated ----
                rts = []
                for r in range(BS):
                    rt = pool.tile(
                        [128, 4 * CW], f32, tag=f"in{r}", bufs=2, name=f"rt{r}_{u}"
                    )
                    nc.sync.dma_start(
                        rt.rearrange("p (g w) -> p g w", g=GPS), xr[ch, sg, r]
                    )
                    rts.append(rt)

                # ---- cast to f16 on Act ----
                bts = []
                for r in range(BS):
                    bt = pool.tile(
                        [128, 4 * CW], f16, tag=f"bc{r}", bufs=2, name=f"bt{r}_{u}"
                    )
                    nc.scalar.copy(bt[:, :], rts[r][:, :])
                    bts.append(bt)
                b0, b1, b2_, b3 = bts

                # ---- vertical sorted-3 across the 4 block rows (DVE) ----
                def vt(name, tag="vq", bufs=1):
                    return pool.tile([128, 4 * CW], f16, tag=tag, bufs=bufs,
                                     name=f"{name}_{u}")

                v1 = vt("v1", "v1")
                tt(v1[:, :], b0[:, :], b1[:, :], MAX)
                w1v = vt("w1v", "w1v")
                tt(w1v[:, :], b0[:, :], b1[:, :], MIN)
                v2 = vt("v2", "v2")
                tt(v2[:, :], b2_[:, :], b3[:, :], MAX)
                w2v = vt("w2v", "w2v")
                tt(w2v[:, :], b2_[:, :], b3[:, :], MIN)
                m = vt("m", "m", bufs=2)
                tt(m[:, :], v1[:, :], v2[:, :], MAX)
                t1 = vt("t1", "t1")
                tt(t1[:, :], v1[:, :], v2[:, :], MIN)
                t2 = vt("t2", "t2")
                tt(t2[:, :], w1v[:, :], w2v[:, :], MAX)
                s2 = vt("s2", "s2", bufs=2)
                tt(s2[:, :], t1[:, :], t2[:, :], MAX)
                t3 = vt("t3", "t3", bufs=2)
                tt(t3[:, :], t1[:, :], t2[:, :], MIN)

                # ---- sum path: 16 f16 matmuls accumulate column sums ----
                # v1+w1v+v2+w2v == b0+b1+b2+b3 elementwise (pair min/max
                # preserves sums), so the PE accumulates exact f16 block
                # column-sums with batch-pair fold + row duplication.
                k0 = sg * NMM
                for ti, vtile in enumerate((v1, w1v, v2, w2v)):
                    for g in range(GPS):
                        k = k0 + ti * GPS + g
                        nc.tensor.matmul(
                            psum_s[:, :], lhsT=ones_sb[:, :],
                            rhs=vtile[:, g * CW : (g + 1) * CW],
                            start=(k == 0), stop=(k == NSG * NMM - 1),
                        )

                # ---- A level: merge column pairs (4 -> 2 per block) ----
                # deint planes m, s2, t3 into even/odd (Act + Pool)
                def deint(src, w_out, par, tag, name, eng):
                    v = src.rearrange("p (g j two) -> p g j two", g=GPS, two=2)
                    t = pool.tile([128, w_out], f16, tag=tag, bufs=1,
                                  name=name + ("e" if par == 0 else "o"))
                    cp = getattr(eng, "copy", None) or eng.tensor_copy
                    cp(t.rearrange("p (g j) -> p g j", g=GPS), v[:, :, :, par])
                    return t

                HW2 = 2 * CW
                me = deint(m, HW2, 0, "me", f"me_{u}", nc.scalar)
                mo = deint(m, HW2, 1, "mo", f"mo_{u}", nc.scalar)
                s2e = deint(s2, HW2, 0, "s2e", f"s2e_{u}", nc.scalar)
                s2o = deint(s2, HW2, 1, "s2o", f"s2o_{u}", nc.scalar)
                t3e = deint(t3, HW2, 0, "t3e", f"t3e_{u}", nc.scalar)
                t3o = deint(t3, HW2, 1, "t3o", f"t3o_{u}", nc.scalar)

                def mid(name):
                    return pool.tile([128, HW2], f16, tag=name, bufs=1,
                                     name=f"{name}_{u}")

                p1 = mid("p1")
                tt(p1[:, :], me[:, :], mo[:, :], MAX)
                u1 = mid("u1")
                tt(u1[:, :], me[:, :], mo[:, :], MIN)
                u2 = mid("u2")
                tt(u2[:, :], s2e[:, :], s2o[:, :], MAX)
                p2 = mid("p2")
                tt(p2[:, :], u1[:, :], u2[:, :], MAX)
                w2 = mid("w2")
                tt(w2[:, :], me[:, :], s2o[:, :], MIN)
                w3 = mid("w3")
                tt(w3[:, :], s2e[:, :], mo[:, :], MIN)
                w4 = mid("w4")
                tt(w4[:, :], w2[:, :], w3[:, :], MAX)
                w1 = mid("w1")
                tt(w1[:, :], t3e[:, :], t3o[:, :], MAX)
                p3 = mid("p3")
                tt(p3[:, :], w1[:, :], w4[:, :], MAX)

                # ---- B level: final merge (2 -> 1 per block) ----
                def deintB(src, par, tag, name, eng):
                    v = src.rearrange("p (g j two) -> p g j two", g=GPS, two=2)
                    t = pool.tile([128, CW], f16, tag=tag, bufs=1,
                                  name=name + ("e" if par == 0 else "o"))
                    cp = getattr(eng, "copy", None) or eng.tensor_copy
                    cp(t.rearrange("p (g j) -> p g j", g=GPS), v[:, :, :, par])
                    return t

                p1e = deintB(p1, 0, "p1e", f"p1e_{u}", nc.scalar)
                p1o = deintB(p1, 1, "p1o", f"p1o_{u}", nc.scalar)
                p2e = deintB(p2, 0, "p2e", f"p2e_{u}", nc.scalar)
                p2o = deintB(p2, 1, "p2o", f"p2o_{u}", nc.scalar)
                p3e = deintB(p3, 0, "p3e", f"p3e_{u}", nc.scalar)
                p3o = deintB(p3, 1, "p3o", f"p3o_{u}", nc.scalar)

                def small(name):
                    return pool.tile([128, CW], f16, tag=name, bufs=1,
                                     name=f"{name}_{u}")

                z1 = small("z1")
                tt(z1[:, :], p1e[:, :], p1o[:, :], MIN)
                z2 = small("z2")
                tt(z2[:, :], p2e[:, :], p2o[:, :], MAX)
                c2 = small("c2")
                tt(c2[:, :], z1[:, :], z2[:, :], MAX)
                z4 = small("z4")
                tt(z4[:, :], p1e[:, :], p2o[:, :], MIN)
                z5 = small("z5")
                tt(z5[:, :], p2e[:, :], p1o[:, :], MIN)
                z6 = small("z6")
                tt(z6[:, :], z4[:, :], z5[:, :], MAX)
                z3 = small("z3")
                tt(z3[:, :], p3e[:, :], p3o[:, :], MAX)
                c3 = small("c3")
                tt(c3[:, :], z3[:, :], z6[:, :], MAX)

                # quantile stat: psum_q += fold(c2) + fold(c3) (the c2+c3
                # add is folded into two PE accumulation matmuls)
                nc.tensor.matmul(
                    psum_q[:, :], lhsT=ones_sb[:, :], rhs=c2[:, :],
                    start=(sg == 0), stop=False,
                )
                nc.tensor.matmul(
                    psum_q[:, :], lhsT=ones_sb[:, :], rhs=c3[:, :],
                    start=False, stop=(sg == NSG - 1),
                )

            # ---- per-chunk tail: fold, EMA update, ratio, broadcast ----
            def tail_tile(name, wdt=CB, dt=f32):
                return pool.tile([128, wdt], dt, tag=name, bufs=2,
                                 name=f"{name}_{ch}")

            # S[p, j] = sum over the 4 columns of each block
            S = tail_tile("S")
            nc.vector.tensor_reduce(
                S[:, :], psum_s.rearrange("p (j c) -> p j c", c=BS),
                mybir.AxisListType.X, ADD,
            )
            # qf[p, j] = sum over the 4 concatenated groups
            qf = tail_tile("qf")
            nc.vector.tensor_reduce(
                qf[:, :], psum_q.rearrange("p (g j) -> p j g", g=GPS),
                mybir.AxisListType.X, ADD,
            )

            ee2 = tail_tile("ee2")
            nc.scalar.activation(
                ee2[:, :], ee_sb[:, :],
                mybir.ActivationFunctionType.Copy, bias=EPS, scale=DECAY,
            )
            eq2 = tail_tile("eq2")
            nc.scalar.activation(
                eq2[:, :], eq_sb[:, :],
                mybir.ActivationFunctionType.Copy, bias=0.0, scale=DECAY,
            )

            den = tail_tile("den")
            nc.vector.scalar_tensor_tensor(
                den[:, :], S[:, :], C_MEAN, ee2[:, :], op0=MULT, op1=ADD
            )
            num = tail_tile("num")
            nc.vector.scalar_tensor_tensor(
                num[:, :], qf[:, :], C_QUANT, eq2[:, :], op0=MULT, op1=ADD
            )
            rec = tail_tile("rec")
            nc.vector.reciprocal(rec[:, :], den[:, :])
            uu = tail_tile("uu")
            nc.vector.tensor_tensor(uu[:, :], num[:, :], rec[:, :], MULT)

            # expand x4 along columns on Act: u4[p, j*4 + c] = uu[p, j]
            u4 = tail_tile("u4", CW)
            u4v = u4.rearrange("p (j c) -> p j c", c=BS)
            for c in range(BS):
                nc.scalar.copy(u4v[:, :, c], uu[:, :])

            # broadcast-source DMAs per (h, r2): dst/src both [64, B, CW]
            u4r = u4.rearrange("(i r2) w -> r2 i w", r2=2)
            for r2 in range(2):
                u4b = u4r[r2].unsqueeze(1).broadcast_to((NBH, B, CW))
                for h in range(2):
                    nc.gpsimd.dma_start(yr[ch, h, r2], u4b)

    _split_multi_waits(nc)
    return nc


def _get_nc():
    if "nc" not in _CACHE:
        _CACHE["nc"] = _build()
    return _CACHE["nc"]


def kernel(current_errors, ema_errors, ema_quantile):
    from concourse.bass_utils import run_bass_kernel_spmd

    x = np.asarray(current_errors, dtype=np.float32).reshape(B, H, W)
    ee = np.asarray(ema_errors, dtype=np.float32).reshape(H // BS, W // BS)
    eq = np.asarray(ema_quantile, dtype=np.float32).reshape(H // BS, W // BS)

    # ones16[p, m] == 1 iff p % 64 == m // 2 (batch-pair fold + row dup)
    ones16 = np.zeros((128, 128), dtype=np.float16)
    p = np.arange(128)
    ones16[p, (p % NBH) * 2] = 1.0
    ones16[p, (p % NBH) * 2 + 1] = 1.0

    in_maps = []
    for k in range(NCORES):
        xs = np.ascontiguousarray(x[:, k * HS : (k + 1) * HS, :]).reshape(ROWS, W)
        ees = np.ascontiguousarray(ee[k * NBH : (k + 1) * NBH, :])
        eqs = np.ascontiguousarray(eq[k * NBH : (k + 1) * NBH, :])
        in_maps.append({"x": xs, "ee": ees, "eq": eqs, "ones16": ones16})

    nc = _get_nc()
    trace = bool(int(os.environ.get("KERNEL_TRACE", "0")))
    try:
        res = run_bass_kernel_spmd(
            nc, in_maps, core_ids=list(range(NCORES)), trace=trace
        )
    except Exception:
        # transient device state (e.g. NRT_EXEC_UNIT_UNRECOVERABLE) — retry once
        res = run_bass_kernel_spmd(
            nc, in_maps, core_ids=list(range(NCORES)), trace=trace
        )
    _CACHE["last_results"] = res

    out = np.empty((B, 1, H, W), dtype=np.float32)
    for k in range(NCORES):
        out[:, 0, k * HS : (k + 1) * HS, :] = res.results[k]["y"].reshape(B, HS, W)
    return out
